# revision 18
# baseline (speedup 1.0000x reference)
"""CRF NLL loss kernel for Trainium2 (8 NeuronCores, batch-sharded).

Strategy: meet-in-the-middle forward/backward recurrences.
----------------------------------------------------------
Data-parallel over batch: each of 8 cores handles 64 sequences.

The log-partition forward algorithm is split into TWO independent chains
that run concurrently, halving the serial mm->mult depth (the dominant
cost -- each step is a PE matmul -> DVE multiply roundtrip of ~0.5us):

  fwd:  alpha_t = exp(f_t) * (E'^T alpha_{t-1}),  t = 0..255
  bwd:  G_{u-1} = E'b (ef_u * (G_u + e_BOS)),     u = 512..256
  Z_b  = sum_p alpha_255[p] * G_255[p]            (one dot at the middle)

E' = exp(trans - C0) with a fixed shift C0 (no rescaling: drift over 256
steps stays within ~e^+-20, far inside fp32/bf16 range; the shift is
undone exactly on the host: logZ_b = log(s_b) + C0*(t*_b + 2)).

Variable lengths (len in [256, 512], so the fwd half is mask-free) are
handled ONLY in the bwd chain via a carrier channel: label BOS is
structurally dead (no transitions into it), so the bwd transition matrix
hijacks column BOS := exp(trans[:, EOS] - C0) and zeroes row BOS.  The
bwd elementwise op is scalar_tensor_tensor: H = (G + e_BOS) * ef, where
ef[BOS, u, b] = exp(host-written 0 / -1e4) = [u == t*_b + 1].  At
u = t*_b+1 the carrier fires and injects exp(trans[:, EOS] - C0) into G
via the hijacked weight column -- exactly the EOS cap of the reference,
at the right per-sequence time.  Before injection G is exactly 0, after
it the indicator is 0, so no masking of real features is ever needed.

Everything is bf16 (features pre-cast on host; 4x faster PE, half DMA);
PSUM accumulation fp32.  Gold path score: host gathers indexed scalars
(index marshalling only), device does the masked weighted sums in fp32,
with the multiplies split into 64-column pieces dropped into the DVE's
per-step idle slack mid-loop and only two reduce_sums + an add at the end.
"""

import numpy as np
import ml_dtypes

B, T, L = 512, 512, 128
NCORES = 8
BC = B // NCORES            # 64 sequences per core
PAD, BOS, EOS = 0, 1, 2
NEG = -10000.0
C0 = 5.9                    # constant per-step log-shift folded into E'
CH = 8                      # steps per feature chunk
M = 255                     # meet point: Z = alpha_M . G_M
NF_STEPS = M + 1            # fwd consumes f_0..f_255   -> 32 chunks
NFC = NF_STEPS // CH
NB_SLOTS = 257              # bwd slots s=0..256 (s=0 is the virtual u=512)
NBC = (NB_SLOTS + CH - 1) // CH  # 33 chunks (last partly padded)

F32 = np.float32
BF16 = ml_dtypes.bfloat16

_compiled = None


def _build():
    import concourse.bass as bass
    import concourse.bacc as bacc
    import concourse.mybir as mybir
    import concourse.tile as tile

    f32 = mybir.dt.float32
    bf16 = mybir.dt.bfloat16
    nc = bacc.Bacc("TRN2", target_bir_lowering=False, debug=False)

    featf = nc.dram_tensor("featf", [NFC, L, CH * BC], bf16, kind="ExternalInput")
    featb = nc.dram_tensor("featb", [NBC, L, CH * BC], bf16, kind="ExternalInput")
    transf = nc.dram_tensor("transf", [L, L], f32, kind="ExternalInput")
    transbT = nc.dram_tensor("transbT", [L, L], f32, kind="ExternalInput")
    emis_v = nc.dram_tensor("emis_v", [BC, T], f32, kind="ExternalInput")
    emis_w = nc.dram_tensor("emis_w", [BC, T], f32, kind="ExternalInput")
    trans_v = nc.dram_tensor("trans_v", [BC, T + 1], f32, kind="ExternalInput")
    trans_w = nc.dram_tensor("trans_w", [BC, T + 1], f32, kind="ExternalInput")
    sbos_in = nc.dram_tensor("sbos_in", [L, 1], f32, kind="ExternalInput")

    sdot_o = nc.dram_tensor("sdot", [1, BC], f32, kind="ExternalOutput")
    gold_o = nc.dram_tensor("gold", [BC, 1], f32, kind="ExternalOutput")

    AX = mybir.AxisListType.X
    MUL = mybir.AluOpType.mult
    ADD = mybir.AluOpType.add
    EXP = mybir.ActivationFunctionType.Exp

    with tile.TileContext(nc) as tc:
        with (
            tc.tile_pool(name="state", bufs=1) as st,
            tc.tile_pool(name="eff", bufs=3) as efp,
            tc.tile_pool(name="efb", bufs=3) as ebp,
            tc.tile_pool(name="ftf", bufs=3) as ffp,
            tc.tile_pool(name="ftb", bufs=3) as fbp,
            tc.tile_pool(name="vfps", bufs=2, space="PSUM") as vfps,
            tc.tile_pool(name="gbps", bufs=2, space="PSUM") as gbps,
            tc.tile_pool(name="sps", bufs=1, space="PSUM") as sps,
            tc.tile_pool(name="misc", bufs=1) as mp,
        ):
            # ---- one-time setup (DMA order = HWDGE order: first-step deps
            # lead -- descriptor generation serializes at ~0.6us/transfer) ----
            nc0 = st.tile([L, 1], f32)          # bias tile: -C0
            nc.vector.memset(nc0[:], -C0)
            zb = st.tile([L, 1], f32)
            nc.vector.memset(zb[:], 0.0)
            zsb = st.tile([L, BC], bf16)        # exact-zero G_512
            nc.vector.memset(zsb[:], 0.0)
            ones_col = st.tile([L, 1], f32)     # lhsT for the final column dots
            nc.vector.memset(ones_col[:], 1.0)

            wf = st.tile([L, 2 * BC], bf16)     # fwd state ping-pong
            hb = st.tile([L, 2 * BC], bf16)     # bwd state ping-pong

            def load_fwd_chunk(c):
                ft = ffp.tile([L, CH * BC], bf16, tag="ftf")
                nc.sync.dma_start(ft[:], featf[c])
                ef = efp.tile([L, CH * BC], bf16, tag="eff")
                nc.scalar.activation(ef[:], ft[:], EXP, bias=zb[:], scale=1.0)
                return ef

            def load_bwd_chunk(c):
                ft = fbp.tile([L, CH * BC], bf16, tag="ftb")
                nc.sync.dma_start(ft[:], featb[c])
                ef = ebp.tile([L, CH * BC], bf16, tag="efb")
                nc.scalar.activation(ef[:], ft[:], EXP, bias=zb[:], scale=1.0)
                return ef

            # fwd-side deps first: trans -> Ef feeds the first matmul,
            # featf0 -> eff + ebos feed the alpha_0 init
            trf_sb = st.tile([L, L], f32)
            nc.sync.dma_start(trf_sb[:], transf[:])
            Ef = st.tile([L, L], bf16)          # lhsT fwd: exp(trans - C0)
            nc.scalar.activation(Ef[:], trf_sb[:], EXP, bias=nc0[:], scale=1.0)
            eff = load_fwd_chunk(0)
            ebos = st.tile([L, 1], f32)         # exp(trans[BOS, :] - C0) column
            nc.sync.dma_start(ebos[:], transf[BOS:BOS + 1, :].rearrange("a b -> b a"))
            nc.scalar.activation(ebos[:], ebos[:], EXP, bias=nc0[:], scale=1.0)
            # alpha_0 = exp(trans[BOS,:] - C0) * exp(f_0)
            nc.vector.tensor_scalar(out=wf[:, 0:BC], in0=eff[:, 0:BC],
                                    scalar1=ebos[:, 0:1], scalar2=None, op0=MUL)

            # bwd-side deps
            efb = load_bwd_chunk(0)
            sbos = st.tile([L, 1], f32)         # e_BOS carrier bias column
            nc.sync.dma_start(sbos[:], sbos_in[:])
            trb_sb = st.tile([L, L], f32)
            nc.sync.dma_start(trb_sb[:], transbT[:])
            EbT = st.tile([L, L], bf16)         # lhsT bwd: exp(trans_b - C0)^T
            nc.scalar.activation(EbT[:], trb_sb[:], EXP, bias=nc0[:], scale=1.0)
            # H_512 = (0 + e_BOS) * ef_512   (carrier only)
            nc.vector.scalar_tensor_tensor(out=hb[:, 0:BC], in0=zsb[:],
                                           scalar=sbos[:, 0:1], in1=efb[:, 0:BC],
                                           op0=ADD, op1=MUL)

            eff_next = load_fwd_chunk(1)
            efb_next = load_bwd_chunk(1)

            # ---- gold score: DMAs at k=64, multiply pieces dropped into the
            # DVE's per-step slack, two reduce_sums + add at the end ----
            gold_tiles = {}
            NP_E = 8            # emis pieces of 64 cols
            NP_T = 8            # trans pieces of 64/65 cols

            def emit_gold_dmas():
                gold_tiles["ev"] = mp.tile([BC, T], f32, tag="gv", name="gold_ev")
                nc.sync.dma_start(gold_tiles["ev"][:], emis_v[:])
                gold_tiles["ew"] = mp.tile([BC, T], f32, tag="gw", name="gold_ew")
                nc.sync.dma_start(gold_tiles["ew"][:], emis_w[:])
                gold_tiles["tv"] = mp.tile([BC, T + 1], f32, tag="tv", name="gold_tv")
                nc.sync.dma_start(gold_tiles["tv"][:], trans_v[:])
                gold_tiles["tw"] = mp.tile([BC, T + 1], f32, tag="tw", name="gold_tw")
                nc.sync.dma_start(gold_tiles["tw"][:], trans_w[:])

            def emit_gold_piece(i):
                # pieces 0..7: emis products in-place; 8..15: trans products
                if i < NP_E:
                    a, b, lo = gold_tiles["ev"], gold_tiles["ew"], i * 64
                    n = 64
                else:
                    a, b, lo = gold_tiles["tv"], gold_tiles["tw"], (i - NP_E) * 64
                    n = 65 if i == NP_E + NP_T - 1 else 64
                nc.vector.tensor_tensor(out=a[:, lo:lo + n], in0=a[:, lo:lo + n],
                                        in1=b[:, lo:lo + n], op=MUL)

            # ---- the two chains, interleaved; k = 1..256 ----
            for k in range(1, NB_SLOTS):
                j = k % CH
                if j == 0:  # rotate chunks, prefetch next
                    c = k // CH
                    eff, efb = eff_next, efb_next
                    if c + 1 < NFC:
                        eff_next = load_fwd_chunk(c + 1)
                    if c + 1 < NBC:
                        efb_next = load_bwd_chunk(c + 1)

                s, sp = (k % 2) * BC, ((k + 1) % 2) * BC

                if k <= M:  # fwd step t = k
                    vF = vfps.tile([L, BC], f32, space="PSUM")
                    nc.tensor.matmul(vF[:], lhsT=Ef[:], rhs=wf[:, sp:sp + BC],
                                     start=True, stop=True)
                    nc.vector.tensor_tensor(out=wf[:, s:s + BC], in0=vF[:],
                                            in1=eff[:, j * BC:(j + 1) * BC], op=MUL)

                # bwd step: G_{512-k} = EbT^T @ H_{513-k}; H_{512-k} via carrier STT
                GB = gbps.tile([L, BC], f32, space="PSUM")
                nc.tensor.matmul(GB[:], lhsT=EbT[:], rhs=hb[:, sp:sp + BC],
                                 start=True, stop=True)
                nc.vector.scalar_tensor_tensor(out=hb[:, s:s + BC], in0=GB[:],
                                               scalar=sbos[:, 0:1],
                                               in1=efb[:, j * BC:(j + 1) * BC],
                                               op0=ADD, op1=MUL)

                if k == 64:
                    emit_gold_dmas()
                if k >= 72 and k % 8 == 0 and (k - 72) // 8 < NP_E + NP_T:
                    emit_gold_piece((k - 72) // 8)

            # ---- epilogue: G_255 and the middle dot ----
            sF = (NB_SLOTS - 1) % 2  # parity of the last-written slot (k=256 -> 0)
            GB = gbps.tile([L, BC], f32, space="PSUM")
            nc.tensor.matmul(GB[:], lhsT=EbT[:], rhs=hb[:, sF * BC:sF * BC + BC],
                             start=True, stop=True)
            prod = mp.tile([L, BC], f32, tag="prod")
            aF = M % 2  # alpha_255 slot parity
            nc.vector.tensor_tensor(out=prod[:], in0=GB[:],
                                    in1=wf[:, aF * BC:aF * BC + BC], op=MUL)
            sdot = sps.tile([1, BC], f32, space="PSUM")
            nc.tensor.matmul(sdot[:], lhsT=ones_col[:], rhs=prod[:],
                             start=True, stop=True)
            sdot_sb = mp.tile([1, BC], f32, tag="sdot_sb")
            nc.vector.tensor_copy(sdot_sb[:], sdot[:])
            nc.sync.dma_start(sdot_o[:], sdot_sb[:])

            # ---- gold total: reduce both product rows, add ----
            g1 = mp.tile([BC, 1], f32, tag="g1")
            nc.vector.reduce_sum(g1[:], gold_tiles["ev"][:], axis=AX)
            g2 = mp.tile([BC, 1], f32, tag="g2")
            nc.vector.reduce_sum(g2[:], gold_tiles["tv"][:], axis=AX)
            nc.vector.tensor_tensor(out=g1[:], in0=g1[:], in1=g2[:], op=ADD)
            nc.sync.dma_start(gold_o[:], g1[:])

    nc.compile()
    return nc


def _get_compiled():
    global _compiled
    if _compiled is None:
        _compiled = _build()
    return _compiled


def _prep_core(feat, tags, maskf, trans_np):
    """Host-side marshalling for one core's shard (indexing/layout only)."""
    lens = maskf.sum(axis=1).astype(np.int64)          # in [T//2, T]
    tstar = lens - 1

    # fwd chunks: featf[c, l, j*BC + b] = feat[b, 8c+j, l], steps 0..255
    ff = feat[:, :NF_STEPS, :].transpose(1, 2, 0)      # [256, L, BC]
    ff = ff.reshape(NFC, CH, L, BC).transpose(0, 2, 1, 3)
    featf = np.ascontiguousarray(ff.reshape(NFC, L, CH * BC)).astype(BF16)

    # bwd slots: slot s holds time u = 512 - s (s=0 virtual, s=1..256 real)
    fb = np.zeros((NBC * CH, BC, L), dtype=F32)        # [264, BC, L]
    fb[1:257] = feat[:, T - 1:M:-1, :].transpose(1, 0, 2)  # u = 511..256
    # carrier row: exp -> indicator [u == t*_b + 1], i.e. slot 511 - t*_b
    fb[:257, :, BOS] = NEG
    fb[T - 1 - tstar, np.arange(BC), BOS] = 0.0
    fbt = fb.reshape(NBC, CH, BC, L).transpose(0, 3, 1, 2)  # [NBC, L, CH, BC]
    featb = np.ascontiguousarray(fbt.reshape(NBC, L, CH * BC)).astype(BF16)

    # bwd transitions: kill row BOS, hijack column BOS := trans[:, EOS]
    trb = trans_np.copy()
    trb[BOS, :] = NEG
    trb[:, BOS] = trans_np[:, EOS]
    trb[BOS, BOS] = NEG
    transbT = np.ascontiguousarray(trb.T)

    # gold inputs (identical to the reference's gather, host-indexed)
    emis_v = np.take_along_axis(feat, tags[..., None], axis=-1)[..., 0]  # [BC,T]
    emis_w = maskf.copy()
    emis_w[:, 0] = 1.0

    trans_v = np.empty((BC, T + 1), dtype=F32)
    trans_v[:, : T - 1] = trans_np[tags[:, :-1], tags[:, 1:]]
    trans_v[:, T - 1] = trans_np[BOS, tags[:, 0]]
    last_lab = tags[np.arange(BC), tstar]
    trans_v[:, T] = trans_np[last_lab, EOS]
    trans_w = np.empty((BC, T + 1), dtype=F32)
    trans_w[:, : T - 1] = maskf[:, 1:]
    trans_w[:, T - 1] = 1.0
    trans_w[:, T] = 1.0

    sbos = np.zeros((L, 1), dtype=F32)
    sbos[BOS, 0] = 1.0

    in_map = {
        "featf": featf,
        "featb": featb,
        "sbos_in": sbos,
        "transf": np.ascontiguousarray(trans_np),
        "transbT": transbT,
        "emis_v": np.ascontiguousarray(emis_v.astype(F32)),
        "emis_w": np.ascontiguousarray(emis_w),
        "trans_v": trans_v,
        "trans_w": trans_w,
    }
    return in_map, tstar


def kernel(features, tag_seqs, mask, transitions):
    from concourse import bass_utils

    feats = np.asarray(features, dtype=F32)
    tags = np.asarray(tag_seqs)
    maskf = np.asarray(mask).astype(F32)
    trans_np = np.asarray(transitions, dtype=F32)

    nc = _get_compiled()

    in_maps, tstars = [], []
    for c in range(NCORES):
        sl = slice(c * BC, (c + 1) * BC)
        m, ts = _prep_core(feats[sl], tags[sl], maskf[sl], trans_np)
        in_maps.append(m)
        tstars.append(ts)

    res = bass_utils.run_bass_kernel_spmd(nc, in_maps, core_ids=list(range(NCORES)))

    per_seq = []
    for c in range(NCORES):
        out = res.results[c]
        ts = tstars[c]
        s = out["sdot"][0, :].astype(np.float64)
        logZ = np.log(s) + C0 * (ts + 2)
        gold = out["gold"][:, 0].astype(np.float64)
        per_seq.append(gold - logZ)

    loss = -np.mean(np.concatenate(per_seq))
    return np.float32(loss)
